# revision 11
# baseline (speedup 1.0000x reference)
"""Cost-volume construction (nn_CostVolume) as a Bass/Trainium2 SPMD kernel.

Problem (hardcoded shapes):
    left_features:  (4, 32, 64, 128) f32
    right_features: (4, 32, 64, 128) f32
    max_disparity:  192  ->  D = 48
    output:         (4, 64, 48, 64, 128) f32
        out[:, :C, d]  = left
        out[:, C:, d, h, w] = right[:, :, h, w+d] if w+d < W else 0

Pure data movement (384 MiB written from 8 MiB of input) -> DMA-only kernel.
Sharding: disparity axis D=48 split 6-per-core across 8 cores.

Key tricks:
- Right half: rows are zero-padded from W=128 to PW=133 and flattened per
  (b,c) partition, so the shifted slab for local disparity ld is exactly
  rext_flat[ld : ld + H*PW]: the shift runs across row boundaries into the
  zero padding, which provides the w+d >= W zero fill; the junk in the
  padding columns is stripped on the host. Every store is one fully
  contiguous ~4.3 MiB DMA.
- Outputs are partition-major ([p, ld, slab]) so every store's DRAM AP is
  2-dim; a channel-major layout gives 3-dim DRAM APs whose descriptors the
  DGE cannot spread across the 16 SDMA engines (measured ~3.4x slower).
- Left and right halves are fully independent chains on the two HWDGE
  rings (SP and ACT), each with its own semaphore, so the two loads and
  the 12 stores overlap.
"""

import numpy as np

import concourse.bass as bass
from concourse import mybir
from concourse.bass_utils import run_bass_kernel_spmd

B, C, H, W = 4, 32, 64, 128
D = 48
NCORES = 8
DLOC = D // NCORES          # 6 disparities per core
PW = W + DLOC - 1           # 133: padded row width (right half)
HW = H * W                  # 8192: left-half slab
SLAB = H * PW               # 8512: right-half slab
SRCW = SLAB + DLOC - 1      # right input per-partition width

_NC_CACHE = {}


def _build(repeat=1):
    """Build the SPMD program. repeat>1 re-runs the whole body that many
    times, serialized per chain on its semaphore — used only for
    steady-state benchmarking; the graded path uses repeat=1."""
    if repeat in _NC_CACHE:
        return _NC_CACHE[repeat]
    nc = bass.Bass()
    left_in = nc.declare_dram_parameter(
        "left", [B * C, HW], mybir.dt.float32, isOutput=False)
    rext_in = nc.declare_dram_parameter(
        "rext", [B * C, SRCW], mybir.dt.float32, isOutput=False)
    out_l = nc.declare_dram_parameter(
        "out_l", [B * C, DLOC, HW], mybir.dt.float32, isOutput=True)
    out_r = nc.declare_dram_parameter(
        "out_r", [B * C, DLOC, SLAB], mybir.dt.float32, isOutput=True)

    with (
        nc.sbuf_tensor([B * C, HW], mybir.dt.float32) as left_t,
        nc.sbuf_tensor([B * C, SRCW], mybir.dt.float32) as rext_t,
        nc.semaphore("sem_l") as sem_l,
        nc.semaphore("sem_r") as sem_r,
        nc.Block() as block,
    ):
        per_iter = 2 * 16

        @block.sync
        def _(sync):
            for r in range(repeat):
                base = per_iter * r
                if r:
                    sync.wait_ge(sem_l, base)
                sync.dma_start(left_t[:], left_in[:]).then_inc(sem_l, 16)
                sync.wait_ge(sem_l, base + 16)
                # one mega-store: broadcast left over DLOC via a step-0 dim
                bsrc = left_t[:, None, :].broadcast_to((B * C, DLOC, HW))
                sync.dma_start(out_l[:, :, :], bsrc).then_inc(sem_l, 16)
            sync.wait_ge(sem_l, per_iter * repeat)

        @block.scalar
        def _(scalar):
            for r in range(repeat):
                base = per_iter * r
                if r:
                    scalar.wait_ge(sem_r, base)
                scalar.dma_start(rext_t[:], rext_in[:]).then_inc(sem_r, 16)
                scalar.wait_ge(sem_r, base + 16)
                # one mega-store: DLOC overlapping shifted windows
                rbase = rext_t[:]
                wsrc = type(rbase)(
                    rbase.tensor, rbase.offset,
                    [list(rbase.ap[0]), [1, DLOC], [1, SLAB]],
                )
                scalar.dma_start(out_r[:, :, :], wsrc).then_inc(sem_r, 16)
            scalar.wait_ge(sem_r, per_iter * repeat)

    _NC_CACHE[repeat] = nc
    return nc


def _host_inputs(left, right):
    """Per-core device input dicts (host-side shard prep)."""
    le_flat = np.ascontiguousarray(left.reshape(B * C, HW))
    rf = right.reshape(B * C, H, W)

    in_maps = []
    for k in range(NCORES):
        d0 = DLOC * k
        re = np.zeros((B * C, H, PW), np.float32)
        take = max(0, W - d0)
        re[:, :, :take] = rf[:, :, d0:d0 + take]
        re_flat = np.zeros((B * C, SRCW), np.float32)
        re_flat[:, :SLAB] = re.reshape(B * C, SLAB)
        in_maps.append({"left": le_flat, "rext": re_flat})
    return in_maps


def _run(in_maps, **kwargs):
    nc = _build()
    return run_bass_kernel_spmd(nc, in_maps, list(range(NCORES)), **kwargs)


def _gather(results):
    out = np.empty((B, 2 * C, D, H, W), np.float32)
    for k in range(NCORES):
        dsl = slice(DLOC * k, DLOC * (k + 1))
        out[:, :C, dsl] = results[k]["out_l"].reshape(B, C, DLOC, H, W)
        slab_r = results[k]["out_r"].reshape(B, C, DLOC, H, PW)
        out[:, C:, dsl] = slab_r[:, :, :, :, :W]
    return out


def kernel(left_features, right_features, max_disparity):
    left = np.asarray(left_features, dtype=np.float32)
    right = np.asarray(right_features, dtype=np.float32)
    assert int(np.asarray(max_disparity)) == 4 * D
    assert left.shape == (B, C, H, W) and right.shape == (B, C, H, W)

    in_maps = _host_inputs(left, right)
    res = _run(in_maps)
    return _gather(res.results)


# revision 14
# speedup vs baseline: 1.0524x; 1.0524x over previous
"""Cost-volume construction (nn_CostVolume) as a Bass/Trainium2 SPMD kernel.

Problem (hardcoded shapes):
    left_features:  (4, 32, 64, 128) f32
    right_features: (4, 32, 64, 128) f32
    max_disparity:  192  ->  D = 48
    output:         (4, 64, 48, 64, 128) f32
        out[:, :C, d]  = left
        out[:, C:, d, h, w] = right[:, :, h, w+d] if w+d < W else 0

Pure data movement (384 MiB written from 8 MiB of input) -> DMA-only kernel.
Sharding: disparity axis D=48 split 6-per-core across 8 cores.

Key tricks:
- Right half: rows are zero-padded from W=128 to PW=133 and flattened per
  (b,c) partition, so the shifted slab for local disparity ld is exactly
  rext_flat[ld : ld + H*PW]: the shift runs across row boundaries into the
  zero padding, which provides the w+d >= W zero fill; the junk in the
  padding columns is stripped on the host. Every store is one fully
  contiguous ~4.3 MiB DMA.
- Outputs are partition-major ([p, ld, slab]) so every store's DRAM AP is
  2-dim; a channel-major layout gives 3-dim DRAM APs whose descriptors the
  DGE cannot spread across the 16 SDMA engines (measured ~3.4x slower).
- Left and right halves are fully independent chains on the two HWDGE
  rings (SP and ACT), each with its own semaphore, so the two loads and
  the 12 stores overlap.
"""

import numpy as np

import concourse.bass as bass
from concourse import mybir
from concourse.bass_utils import run_bass_kernel_spmd

B, C, H, W = 4, 32, 64, 128
D = 48
NCORES = 8
DLOC = D // NCORES          # 6 disparities per core
PW = W + DLOC - 1           # 133: padded row width (right half)
HW = H * W                  # 8192: left-half slab
SLAB = H * PW               # 8512: right-half slab
SRCW = SLAB + DLOC - 1      # right input per-partition width

_NC_CACHE = {}


def _build(repeat=1, variant="mega"):
    """Build the SPMD program. repeat>1 re-runs the whole body that many
    times, serialized per chain on its semaphore — used only for
    steady-state benchmarking; the graded path uses repeat=1.
    variant: "mega" = one store per half; "multi" = one store per ld."""
    if (repeat, variant) in _NC_CACHE:
        return _NC_CACHE[(repeat, variant)]
    nc = bass.Bass()
    left_in = nc.declare_dram_parameter(
        "left", [B * C, HW], mybir.dt.float32, isOutput=False)
    rext_in = nc.declare_dram_parameter(
        "rext", [B * C, SRCW], mybir.dt.float32, isOutput=False)
    out_l = nc.declare_dram_parameter(
        "out_l", [B * C, DLOC, HW], mybir.dt.float32, isOutput=True)
    out_r = nc.declare_dram_parameter(
        "out_r", [B * C, DLOC, SLAB], mybir.dt.float32, isOutput=True)

    with (
        nc.sbuf_tensor([B * C, HW], mybir.dt.float32) as left_t,
        nc.sbuf_tensor([B * C, SRCW], mybir.dt.float32) as rext_t,
        nc.semaphore("sem_l") as sem_l,
        nc.semaphore("sem_r") as sem_r,
        nc.Block() as block,
    ):
        n_store = 1 if variant == "mega" else DLOC
        per_iter = (1 + n_store) * 16

        @block.sync
        def _(sync):
            for r in range(repeat):
                base = per_iter * r
                if r:
                    sync.wait_ge(sem_l, base)
                sync.dma_start(left_t[:], left_in[:]).then_inc(sem_l, 16)
                sync.wait_ge(sem_l, base + 16)
                if variant == "mega":
                    # one store: broadcast left over DLOC via a step-0 dim
                    bsrc = left_t[:, None, :].broadcast_to((B * C, DLOC, HW))
                    sync.dma_start(out_l[:, :, :], bsrc).then_inc(sem_l, 16)
                else:
                    for ld in range(DLOC):
                        sync.dma_start(
                            out_l[:, ld, :], left_t[:]
                        ).then_inc(sem_l, 16)
            sync.wait_ge(sem_l, per_iter * repeat)

        @block.scalar
        def _(scalar):
            for r in range(repeat):
                base = per_iter * r
                if r:
                    scalar.wait_ge(sem_r, base)
                scalar.dma_start(rext_t[:], rext_in[:]).then_inc(sem_r, 16)
                scalar.wait_ge(sem_r, base + 16)
                if variant == "mega":
                    # one store: DLOC overlapping shifted windows
                    rbase = rext_t[:]
                    wsrc = type(rbase)(
                        rbase.tensor, rbase.offset,
                        [list(rbase.ap[0]), [1, DLOC], [1, SLAB]],
                    )
                    scalar.dma_start(out_r[:, :, :], wsrc).then_inc(sem_r, 16)
                else:
                    for ld in range(DLOC):
                        scalar.dma_start(
                            out_r[:, ld, :], rext_t[:, ld:ld + SLAB]
                        ).then_inc(sem_r, 16)
            scalar.wait_ge(sem_r, per_iter * repeat)

    _NC_CACHE[(repeat, variant)] = nc
    return nc


def _host_inputs(left, right):
    """Per-core device input dicts (host-side shard prep)."""
    le_flat = np.ascontiguousarray(left.reshape(B * C, HW))
    rf = right.reshape(B * C, H, W)

    in_maps = []
    for k in range(NCORES):
        d0 = DLOC * k
        re = np.zeros((B * C, H, PW), np.float32)
        take = max(0, W - d0)
        re[:, :, :take] = rf[:, :, d0:d0 + take]
        re_flat = np.zeros((B * C, SRCW), np.float32)
        re_flat[:, :SLAB] = re.reshape(B * C, SLAB)
        in_maps.append({"left": le_flat, "rext": re_flat})
    return in_maps


def _run(in_maps, **kwargs):
    nc = _build()
    return run_bass_kernel_spmd(nc, in_maps, list(range(NCORES)), **kwargs)


def _gather(results):
    out = np.empty((B, 2 * C, D, H, W), np.float32)
    for k in range(NCORES):
        dsl = slice(DLOC * k, DLOC * (k + 1))
        out[:, :C, dsl] = results[k]["out_l"].reshape(B, C, DLOC, H, W)
        slab_r = results[k]["out_r"].reshape(B, C, DLOC, H, PW)
        out[:, C:, dsl] = slab_r[:, :, :, :, :W]
    return out


def kernel(left_features, right_features, max_disparity):
    left = np.asarray(left_features, dtype=np.float32)
    right = np.asarray(right_features, dtype=np.float32)
    assert int(np.asarray(max_disparity)) == 4 * D
    assert left.shape == (B, C, H, W) and right.shape == (B, C, H, W)

    in_maps = _host_inputs(left, right)
    res = _run(in_maps)
    return _gather(res.results)


# revision 19
# speedup vs baseline: 2.8873x; 2.7434x over previous
"""Cost-volume construction (nn_CostVolume) as a Bass/Trainium2 SPMD kernel.

Problem (hardcoded shapes):
    left_features:  (4, 32, 64, 128) f32
    right_features: (4, 32, 64, 128) f32
    max_disparity:  192  ->  D = 48
    output:         (4, 64, 48, 64, 128) f32
        out[:, :C, d]  = left
        out[:, C:, d, h, w] = right[:, :, h, w+d] if w+d < W else 0

Pure data movement (384 MiB written from 8 MiB of input) -> DMA-only kernel.
Sharding: disparity axis D=48 split 6-per-core across 8 cores.

Key tricks:
- Right half: rows are zero-padded from W=128 to PW=133 and flattened per
  (b,c) partition, so the shifted slab for local disparity ld is exactly
  rext_flat[ld : ld + H*PW]: the shift runs across row boundaries into the
  zero padding, which provides the w+d >= W zero fill; the junk in the
  padding columns is stripped on the host. Every store is one fully
  contiguous ~4.3 MiB DMA.
- Outputs are partition-major ([p, ld, slab]) so every store's DRAM AP is
  2-dim; a channel-major layout gives 3-dim DRAM APs whose descriptors the
  DGE cannot spread across the 16 SDMA engines (measured ~3.4x slower).
- Left and right halves are fully independent chains on the two HWDGE
  rings (SP and ACT), each with its own semaphore, so the two loads and
  the 12 stores overlap.
"""

import numpy as np

import concourse.bass as bass
from concourse import mybir
from concourse.bass_utils import run_bass_kernel_spmd

B, C, H, W = 4, 32, 64, 128
D = 48
NCORES = 8
DLOC = D // NCORES          # 6 disparities per core
PW = W + DLOC - 1           # 133: padded row width (right half)
HW = H * W                  # 8192: left-half slab
SLAB = H * PW               # 8512: right-half slab
SRCW = SLAB + DLOC - 1      # right input per-partition width

_NC_CACHE = {}


def _build(repeat=1, variant="mega"):
    """Build the SPMD program. repeat>1 re-runs the whole body that many
    times, serialized per chain on its semaphore — used only for
    steady-state benchmarking; the graded path uses repeat=1.
    variant: "mega" = one store per half; "multi" = one store per ld."""
    if (repeat, variant) in _NC_CACHE:
        return _NC_CACHE[(repeat, variant)]
    nc = bass.Bass()
    left_in = nc.declare_dram_parameter(
        "left", [B * C, HW], mybir.dt.float32, isOutput=False)
    rext_in = nc.declare_dram_parameter(
        "rext", [B * C, SRCW], mybir.dt.float32, isOutput=False)
    out_l = nc.declare_dram_parameter(
        "out_l", [B * C, DLOC, HW], mybir.dt.float32, isOutput=True)
    out_r = nc.declare_dram_parameter(
        "out_r", [B * C, DLOC, SLAB], mybir.dt.float32, isOutput=True)

    if variant == "pipe":
        nc = _build_pipe(nc, repeat, left_in, rext_in, out_l, out_r)
        _NC_CACHE[(repeat, variant)] = nc
        return nc

    with (
        nc.sbuf_tensor([B * C, HW], mybir.dt.float32) as left_t,
        nc.sbuf_tensor([B * C, SRCW], mybir.dt.float32) as rext_t,
        nc.semaphore("sem_l") as sem_l,
        nc.semaphore("sem_r") as sem_r,
        nc.Block() as block,
    ):
        n_store = 1 if variant == "mega" else DLOC
        per_iter = (1 + n_store) * 16

        @block.sync
        def _(sync):
            for r in range(repeat):
                base = per_iter * r
                if r:
                    sync.wait_ge(sem_l, base)
                sync.dma_start(left_t[:], left_in[:]).then_inc(sem_l, 16)
                sync.wait_ge(sem_l, base + 16)
                if variant == "mega":
                    # one store: broadcast left over DLOC via a step-0 dim
                    bsrc = left_t[:, None, :].broadcast_to((B * C, DLOC, HW))
                    sync.dma_start(out_l[:, :, :], bsrc).then_inc(sem_l, 16)
                else:
                    for ld in range(DLOC):
                        sync.dma_start(
                            out_l[:, ld, :], left_t[:]
                        ).then_inc(sem_l, 16)
            sync.wait_ge(sem_l, per_iter * repeat)

        @block.scalar
        def _(scalar):
            for r in range(repeat):
                base = per_iter * r
                if r:
                    scalar.wait_ge(sem_r, base)
                scalar.dma_start(rext_t[:], rext_in[:]).then_inc(sem_r, 16)
                scalar.wait_ge(sem_r, base + 16)
                if variant == "mega":
                    # one store: DLOC overlapping shifted windows
                    rbase = rext_t[:]
                    wsrc = type(rbase)(
                        rbase.tensor, rbase.offset,
                        [list(rbase.ap[0]), [1, DLOC], [1, SLAB]],
                    )
                    scalar.dma_start(out_r[:, :, :], wsrc).then_inc(sem_r, 16)
                else:
                    for ld in range(DLOC):
                        scalar.dma_start(
                            out_r[:, ld, :], rext_t[:, ld:ld + SLAB]
                        ).then_inc(sem_r, 16)
            scalar.wait_ge(sem_r, per_iter * repeat)

    _NC_CACHE[(repeat, variant)] = nc
    return nc


def _build_pipe(nc, repeat, left_in, rext_in, out_l, out_r):
    """Chunked load->store pipeline: loads stream on the gpsimd (SWDGE)
    ring in column quarters; each half's store chain consumes quarters as
    they land, so writes overlap the tail of the reads."""
    Q = 4
    LQ = HW // Q            # 2048 left cols per quarter
    RQ = SLAB // Q          # 2128 right cols per quarter (4*2128+5 = SRCW)
    with (
        nc.sbuf_tensor([B * C, HW], mybir.dt.float32) as left_t,
        nc.sbuf_tensor([B * C, SRCW], mybir.dt.float32) as rext_t,
        nc.semaphore("sst_l") as sst_l,
        nc.semaphore("sst_r") as sst_r,
        nc.Block() as block,
    ):
        # one sem per load quarter: a single DMA inc per iteration, and
        # consumers wait on the full value — intermediate thresholds on a
        # multi-inc sem are racy (per-engine slice completion interleaves).
        sld_l = [nc.alloc_semaphore(f"sld_l{q}") for q in range(Q)]
        sld_r = [nc.alloc_semaphore(f"sld_r{q}") for q in range(Q)]
        per_st = Q * 16

        @block.gpsimd
        def _(gpsimd):
            for r in range(repeat):
                if r:
                    gpsimd.wait_ge(sst_l, per_st * r)
                    gpsimd.wait_ge(sst_r, per_st * r)
                for q in range(Q):
                    gpsimd.dma_start(
                        left_t[:, q * LQ:(q + 1) * LQ],
                        left_in[:, q * LQ:(q + 1) * LQ],
                    ).then_inc(sld_l[q], 16)
                    # right quarter includes the +DLOC-1 tail on the last one
                    w = RQ if q < Q - 1 else RQ + DLOC - 1
                    gpsimd.dma_start(
                        rext_t[:, q * RQ:q * RQ + w],
                        rext_in[:, q * RQ:q * RQ + w],
                    ).then_inc(sld_r[q], 16)

        @block.sync
        def _(sync):
            for r in range(repeat):
                for q in range(Q):
                    sync.wait_ge(sld_l[q], 16 * (r + 1))
                    lbase = left_t[:]
                    src = type(lbase)(
                        lbase.tensor, lbase.offset + q * LQ,
                        [list(lbase.ap[0]), [0, DLOC], [1, LQ]],
                    )
                    dst = type(out_l[:])(
                        out_l[:].tensor, q * LQ,
                        [[DLOC * HW, B * C], [HW, DLOC], [1, LQ]],
                    )
                    sync.dma_start(dst, src).then_inc(sst_l, 16)
            sync.wait_ge(sst_l, per_st * repeat)

        @block.scalar
        def _(scalar):
            for r in range(repeat):
                for q in range(Q):
                    # store quarter q reads src cols [ld+q*RQ, ld+q*RQ+RQ);
                    # ld<DLOC spills DLOC-1 cols into quarter q+1, so wait
                    # for that quarter too (the last quarter's spill is
                    # covered by the widened final load).
                    scalar.wait_ge(sld_r[q], 16 * (r + 1))
                    if q < Q - 1:
                        scalar.wait_ge(sld_r[q + 1], 16 * (r + 1))
                    rbase = rext_t[:]
                    src = type(rbase)(
                        rbase.tensor, rbase.offset + q * RQ,
                        [list(rbase.ap[0]), [1, DLOC], [1, RQ]],
                    )
                    dst = type(out_r[:])(
                        out_r[:].tensor, q * RQ,
                        [[DLOC * SLAB, B * C], [SLAB, DLOC], [1, RQ]],
                    )
                    scalar.dma_start(dst, src).then_inc(sst_r, 16)
            scalar.wait_ge(sst_r, per_st * repeat)

    return nc


def _host_inputs(left, right):
    """Per-core device input dicts (host-side shard prep)."""
    le_flat = np.ascontiguousarray(left.reshape(B * C, HW))
    rf = right.reshape(B * C, H, W)

    in_maps = []
    for k in range(NCORES):
        d0 = DLOC * k
        re = np.zeros((B * C, H, PW), np.float32)
        take = max(0, W - d0)
        re[:, :, :take] = rf[:, :, d0:d0 + take]
        re_flat = np.zeros((B * C, SRCW), np.float32)
        re_flat[:, :SLAB] = re.reshape(B * C, SLAB)
        in_maps.append({"left": le_flat, "rext": re_flat})
    return in_maps


def _run(in_maps, **kwargs):
    nc = _build()
    return run_bass_kernel_spmd(nc, in_maps, list(range(NCORES)), **kwargs)


def _gather(results):
    out = np.empty((B, 2 * C, D, H, W), np.float32)
    for k in range(NCORES):
        dsl = slice(DLOC * k, DLOC * (k + 1))
        out[:, :C, dsl] = results[k]["out_l"].reshape(B, C, DLOC, H, W)
        slab_r = results[k]["out_r"].reshape(B, C, DLOC, H, PW)
        out[:, C:, dsl] = slab_r[:, :, :, :, :W]
    return out


def kernel(left_features, right_features, max_disparity):
    left = np.asarray(left_features, dtype=np.float32)
    right = np.asarray(right_features, dtype=np.float32)
    assert int(np.asarray(max_disparity)) == 4 * D
    assert left.shape == (B, C, H, W) and right.shape == (B, C, H, W)

    in_maps = _host_inputs(left, right)
    res = _run(in_maps)
    return _gather(res.results)
